# revision 10
# baseline (speedup 1.0000x reference)
"""Trainium2 Bass kernel for nn_AutoregressiveRoutingHead.

Model (per batch row b):
    tok_in = [START, tgt[0..6]]                       # teacher forcing, START=5
    x_t    = emb[tok_in[t]]                           # (HID,)
    gi     = x_t @ W_ih.T + b_ih                      # (768,)
    gh     = h @ W_hh.T + b_hh                        # (768,)
    r = sigmoid(gi_r + gh_r); z = sigmoid(gi_z + gh_z)
    n = tanh(gi_n + r * gh_n)
    h' = (1-z)*n + z*h = n - z*(n - h)
    logits_t = h' @ W_out.T + b_out                   # (5,)

Strategy: pure data parallel over batch (65536 -> 8 x 8192). On each core the
hidden state lives TRANSPOSED (latent dim on partitions, batch on the free dim)
so the recurrence matmul needs no per-step transposes. The embedding gather is
a K=8 onehot matmul accumulated into the same PSUM as the recurrence matmul.
Step 0's input is the constant START embedding, folded in as per-partition
activation biases (no matmul at all). Batch is processed in column chunks;
PSUM frames are half-size and double-buffered so one chunk's matmuls overlap
the previous chunk's elementwise work (keeps the PE HAM-warm).
"""

import numpy as np

import concourse.bass as bass
import concourse.mybir as mybir
import concourse.tile as tile
from concourse import bacc, bass_utils

F32 = mybir.dt.float32
AF = mybir.ActivationFunctionType
ALU = mybir.AluOpType

N_CORES = 8
B = 65536
L = 8
LATENT = 256
HID = 128
NTOK = 5
V = NTOK + 1  # vocab incl <start>
START = NTOK
G = 3 * LATENT  # 768 gate rows
KC = LATENT // 128  # 2 contraction chunks
MC = G // 128  # 6 gate-row chunks

B_CORE = B // N_CORES


def build_program(b_core=B_CORE, n_b=256, use_bhh_n=False, use_bout=False, mm="f16"):
    """Build + compile the per-core Bass program (SPMD: same program, 8 cores)."""
    nc = bacc.Bacc("TRN2", target_bir_lowering=False, debug=False)
    if mm == "f32":
        DT = F32
    elif mm == "f32r":
        DT = mybir.dt.float32r
    else:
        DT = mybir.dt.float16  # matmul-input + gate dtype
    n_chunks = b_core // n_b
    n_p = n_b // 128  # 128-row blocks per chunk (for the h0 transpose)

    # ---- DRAM I/O ----------------------------------------------------------
    lat = nc.dram_tensor("lat", [b_core, LATENT], F32, kind="ExternalInput").ap()
    # tokrep[j, t-1, b] = tok_in[b, t] for all j (compare rows 6,7 give 0)
    tokrep = nc.dram_tensor("tokrep", [8, L - 1, b_core], F32, kind="ExternalInput").ap()
    embT = nc.dram_tensor("embT", [HID, V], F32, kind="ExternalInput").ap()
    wihT = nc.dram_tensor("wihT", [HID, G], F32, kind="ExternalInput").ap()
    # row 0: b_ih ; row 1: b_hh with the n-part zeroed (rz part only)
    brows = nc.dram_tensor("brows", [2, G], F32, kind="ExternalInput").ap()
    whhT = nc.dram_tensor("whhT", [LATENT, G], DT, kind="ExternalInput").ap()
    woutT = nc.dram_tensor("woutT", [LATENT, NTOK], DT, kind="ExternalInput").ap()
    ident = nc.dram_tensor("ident", [128, 128], F32, kind="ExternalInput").ap()
    iota = nc.dram_tensor("iota", [8, 1], F32, kind="ExternalInput").ap()
    bhhn = bout = None
    if use_bhh_n:
        bhhn = nc.dram_tensor("bhhn", [1, LATENT], DT, kind="ExternalInput").ap()
    if use_bout:
        bout = nc.dram_tensor("bout", [1, NTOK], DT, kind="ExternalInput").ap()
    outT = nc.dram_tensor("outT", [L, NTOK, b_core], F32, kind="ExternalOutput").ap()

    with tile.TileContext(nc) as tc:
        with tc.tile_pool(name="singles", bufs=1) as singles, \
             tc.tile_pool(name="lat_in", bufs=2) as lat_pool, \
             tc.tile_pool(name="hpool", bufs=3) as h_pool, \
             tc.tile_pool(name="gates", bufs=3) as g_pool, \
             tc.tile_pool(name="psum", bufs=1, space="PSUM") as ps:

            # ---- constants / weights in SBUF -------------------------------
            id_sb = singles.tile([128, 128], F32, tag="ident")
            nc.sync.dma_start(id_sb, ident)
            whh_sb = singles.tile([128, KC, G], DT, tag="whh")
            nc.sync.dma_start(whh_sb, whhT.rearrange("(kc p) n -> p kc n", p=128))
            wout_sb = singles.tile([128, KC, NTOK], DT, tag="wout")
            nc.sync.dma_start(wout_sb, woutT.rearrange("(kc p) n -> p kc n", p=128))
            embT_sb = singles.tile([HID, V], F32, tag="embT")
            nc.sync.dma_start(embT_sb, embT)
            wih_sb = singles.tile([HID, G], F32, tag="wih")
            nc.sync.dma_start(wih_sb, wihT)
            brows_sb = singles.tile([2, G], F32, tag="brows")
            nc.sync.dma_start(brows_sb, brows)
            ones2 = singles.tile([2, V], F32, tag="ones2")
            nc.vector.memset(ones2, 1.0)
            iota_sb = singles.tile([8, 1], F32, tag="iota")
            nc.sync.dma_start(iota_sb, iota)
            bhhn_sb = bout_sb = ones_row = None
            if use_bhh_n or use_bout:
                ones_row = singles.tile([1, n_b], DT, tag="ones_row")
                nc.vector.memset(ones_row, 1.0)
            if use_bhh_n:
                bhhn_sb = singles.tile([1, LATENT], DT, tag="bhhn")
                nc.sync.dma_start(bhhn_sb, bhhn)
            if use_bout:
                bout_sb = singles.tile([1, NTOK], DT, tag="bout")
                nc.sync.dma_start(bout_sb, bout)

            # gi table: gi[v, :] = emb[v] @ W_ih.T + b_ih (+ b_hh on rz part)
            # rows 6,7 zero so the per-step gather matmul can use K=8.
            gi_sb = singles.tile([8, G], DT, tag="gi")
            nc.vector.memset(gi_sb, 0.0)
            gp_a = ps.tile([V, 512], F32, tag="rz0")
            nc.tensor.matmul(gp_a, lhsT=embT_sb, rhs=wih_sb[:, 0:512], start=True, stop=False)
            nc.tensor.matmul(gp_a, lhsT=ones2, rhs=brows_sb[:, 0:512], start=False, stop=True)
            gp_b = ps.tile([V, G - 512], F32, tag="in0")
            nc.tensor.matmul(gp_b, lhsT=embT_sb, rhs=wih_sb[:, 512:G], start=True, stop=False)
            nc.tensor.matmul(gp_b, lhsT=ones2[0:1], rhs=brows_sb[0:1, 512:G], start=False, stop=True)
            nc.any.tensor_copy(gi_sb[0:V, 0:512], gp_a)
            nc.any.tensor_copy(gi_sb[0:V, 512:G], gp_b)
            gi32_sb = gi_sb
            if DT != F32:
                gi32_sb = singles.tile([V, G], F32, tag="gi32")
                nc.any.tensor_copy(gi32_sb[:, 0:512], gp_a)
                nc.any.tensor_copy(gi32_sb[:, 512:G], gp_b)

            # transposed gi chunks (for step-0 constant-START biases)
            giT_sb = singles.tile([128, MC, V], F32, tag="giT")
            for m in range(MC):
                tp = ps.tile([128, V], F32, tag="hn0")
                nc.tensor.transpose(tp, gi32_sb[0:V, m * 128:(m + 1) * 128], id_sb[0:V, 0:V])
                nc.any.tensor_copy(giT_sb[:, m, :], tp)

            # ---- main loop: process chunks in PAIRS, steps interleaved,
            # with parity-split PSUM tags (4 banks per parity) so one chunk's
            # matmuls overlap the partner chunk's elementwise chain. ----------
            def chunk_prologue(c, par):
                cs = slice(c * n_b, (c + 1) * n_b)
                lat_sb = lat_pool.tile([128, n_p, LATENT], F32, tag=f"lat{par}",
                                       name="lat_sb")
                nc.sync.dma_start(lat_sb, lat[cs, :].rearrange("(q p) d -> p q d", p=128))
                tokc = lat_pool.tile([8, L - 1, n_b], F32, tag=f"tokc{par}", name="tokc")
                nc.sync.dma_start(tokc, tokrep[:, :, cs])
                oh_c = lat_pool.tile([8, L - 1, n_b], DT, tag=f"ohc{par}", name="oh_c")
                nc.vector.tensor_scalar(oh_c, tokc, iota_sb, None, op0=ALU.is_equal)
                h = h_pool.tile([128, KC, n_b], DT, tag=f"h{par}", name="h0")
                for k in range(KC):
                    for q in range(n_p):
                        tp = ps.tile([128, 128], F32, tag=f"hn{par}", name="tp")
                        nc.tensor.transpose(tp, lat_sb[:, q, k * 128:(k + 1) * 128], id_sb)
                        nc.vector.tensor_copy(h[:, k, q * 128:(q + 1) * 128], tp)
                return cs, oh_c, h

            def step_body(t, par, cs, oh_c, h):
                # ---- matmuls into PSUM ----
                # rz4: [:, m, :] = gate-row chunk m (r0,r1,z0,z1); 2 banks
                rz4 = ps.tile([128, 4, n_b], F32, tag=f"rz{par}", name="rz4")
                hn2 = ps.tile([128, 2, n_b], F32, tag=f"hn{par}", name="hn2")
                in2 = None
                if t > 0:
                    in2 = ps.tile([128, 2, n_b], F32, tag=f"in{par}", name="in2")
                oh = None if t == 0 else oh_c[:, t - 1, :]

                # gi matmuls lead each bank's accumulation group (they only
                # need the onehot, not h'); the partner chunk keeps the PE fed
                # during this chunk's elementwise chain.
                if t > 0:
                    for j in range(2):
                        nc.tensor.matmul(
                            in2[:, j, :], lhsT=gi_sb[:, (4 + j) * 128:(5 + j) * 128],
                            rhs=oh, start=True, stop=True)
                for m in range(MC):
                    tgt = rz4[:, m, :] if m < 4 else hn2[:, m - 4, :]
                    has_gi = (t > 0) and (m < 4)
                    extra_b = (m >= 4) and use_bhh_n
                    if has_gi:
                        nc.tensor.matmul(
                            tgt, lhsT=gi_sb[:, m * 128:(m + 1) * 128],
                            rhs=oh, start=True, stop=False)
                    for k in range(KC):
                        nc.tensor.matmul(
                            tgt,
                            lhsT=whh_sb[:, k, m * 128:(m + 1) * 128],
                            rhs=h[:, k, :],
                            start=(k == 0) and not has_gi,
                            stop=(k == KC - 1) and not extra_b)
                    if extra_b:
                        nc.tensor.matmul(
                            tgt, lhsT=bhhn_sb[:, (m - 4) * 128:(m - 3) * 128],
                            rhs=ones_row, start=False, stop=True)

                # ---- elementwise ----
                rz_sig = g_pool.tile([128, 4, n_b], DT, tag=f"rz_sig{par}", name="rz_sig")
                if t == 0:
                    for m in range(4):
                        nc.scalar.activation(
                            rz_sig[:, m, :], rz4[:, m, :], AF.Sigmoid,
                            bias=giT_sb[:, m, START:START + 1])
                else:
                    nc.scalar.activation(rz_sig, rz4, AF.Sigmoid)
                r = rz_sig[:, 0:2, :]
                z = rz_sig[:, 2:4, :]
                p = g_pool.tile([128, 2, n_b], DT, tag=f"p{par}", name="p")
                nc.vector.tensor_mul(p, r, hn2)
                npre = g_pool.tile([128, 2, n_b], DT, tag=f"npre{par}", name="npre")
                if t == 0:
                    for j in range(2):
                        nc.vector.tensor_scalar_add(
                            npre[:, j, :], p[:, j, :], giT_sb[:, 4 + j, START:START + 1])
                else:
                    nc.vector.tensor_add(npre, p, in2)
                nt = g_pool.tile([128, 2, n_b], DT, tag=f"nt{par}", name="nt")
                nc.scalar.activation(nt, npre, AF.Tanh)

                h_new = h_pool.tile([128, KC, n_b], DT, tag=f"h{par}", name="h_new")
                d = g_pool.tile([128, 2, n_b], DT, tag=f"d{par}", name="d")
                e = g_pool.tile([128, 2, n_b], DT, tag=f"e{par}", name="e")
                # h' = n - z*(n - h)
                nc.vector.tensor_tensor(d, nt, h, ALU.subtract)
                nc.vector.tensor_mul(e, z, d)
                nc.vector.tensor_tensor(h_new, nt, e, ALU.subtract)

                # ---- logits ----
                lg = ps.tile([NTOK, n_b], F32, tag=f"in{par}", name="lg")
                for k in range(KC):
                    nc.tensor.matmul(
                        lg, lhsT=wout_sb[:, k, :], rhs=h_new[:, k, :],
                        start=(k == 0), stop=(k == KC - 1) and not use_bout)
                if use_bout:
                    nc.tensor.matmul(lg, lhsT=bout_sb, rhs=ones_row, start=False, stop=True)
                lg_sb = g_pool.tile([NTOK, n_b], F32, tag=f"lg{par}", name="lg_sb")
                nc.scalar.copy(lg_sb, lg)
                nc.sync.dma_start(outT[t, :, cs], lg_sb)
                return h_new

            assert n_chunks % 2 == 0
            for cpair in range(n_chunks // 2):
                states = []
                for par in range(2):
                    states.append(chunk_prologue(2 * cpair + par, par))
                for t in range(L):
                    for par in range(2):
                        cs, oh_c, h = states[par]
                        h_new = step_body(t, par, cs, oh_c, h)
                        states[par] = (cs, oh_c, h_new)

    nc.compile()
    return nc


def make_in_maps(latent_context, target_sequence, emb_table, W_ih, W_hh,
                 b_ih, b_hh, W_out, b_out, b_core=B_CORE, mm="f16"):
    """Shard + lay out the inputs for each core. Layout-only host transforms."""
    lat = np.ascontiguousarray(np.asarray(latent_context, dtype=np.float32))
    tok = np.asarray(target_sequence).astype(np.float32)
    embT = np.ascontiguousarray(np.asarray(emb_table, dtype=np.float32).T)
    wihT = np.ascontiguousarray(np.asarray(W_ih, dtype=np.float32).T)
    wdt = np.float32 if mm in ("f32", "f32r") else np.float16
    whhT = np.ascontiguousarray(np.asarray(W_hh, dtype=np.float32).T.astype(wdt))
    woutT = np.ascontiguousarray(np.asarray(W_out, dtype=np.float32).T.astype(wdt))
    b_ih = np.asarray(b_ih, dtype=np.float32)
    b_hh = np.asarray(b_hh, dtype=np.float32)
    b_out = np.asarray(b_out, dtype=np.float32)

    brows = np.zeros((2, G), np.float32)
    brows[0] = b_ih
    brows[1, :512] = b_hh[:512]  # n-part of b_hh handled separately
    ident = np.eye(128, dtype=np.float32)
    iota = np.arange(8, dtype=np.float32).reshape(-1, 1)

    n_cores_eff = lat.shape[0] // b_core
    in_maps = []
    for i in range(n_cores_eff):
        sl = slice(i * b_core, (i + 1) * b_core)
        # tokrep[j, t-1, b] = tok_in[b, t] (same for all j)
        tokrep = np.broadcast_to(tok[sl, :L - 1].T[None, :, :], (8, L - 1, b_core))
        m = {
            "lat": lat[sl],
            "tokrep": np.ascontiguousarray(tokrep),
            "embT": embT,
            "wihT": wihT,
            "brows": brows,
            "whhT": whhT,
            "woutT": woutT,
            "ident": ident,
            "iota": iota,
        }
        if np.any(b_hh[512:]):
            m["bhhn"] = np.ascontiguousarray(b_hh[512:].reshape(1, LATENT).astype(wdt))
        if np.any(b_out):
            m["bout"] = np.ascontiguousarray(b_out.reshape(1, NTOK).astype(wdt))
        in_maps.append(m)
    return in_maps


_PROGRAM_CACHE = {}


def _get_program(b_core, use_bhh_n, use_bout, mm):
    key = (b_core, use_bhh_n, use_bout, mm)
    if key not in _PROGRAM_CACHE:
        _PROGRAM_CACHE[key] = build_program(
            b_core=b_core, use_bhh_n=use_bhh_n, use_bout=use_bout, mm=mm)
    return _PROGRAM_CACHE[key]


def run(inputs, trace=False, b_core=B_CORE, mm="f16"):
    in_maps = make_in_maps(b_core=b_core, mm=mm, **inputs)
    use_bhh_n = "bhhn" in in_maps[0]
    use_bout = "bout" in in_maps[0]
    nc = _get_program(b_core, use_bhh_n, use_bout, mm)
    core_ids = list(range(len(in_maps)))
    res = bass_utils.run_bass_kernel_spmd(nc, in_maps, core_ids, trace=trace)
    outs = []
    for i in core_ids:
        o = res.results[i]["outT"]  # (L, NTOK, b_core)
        outs.append(np.ascontiguousarray(np.transpose(o, (2, 0, 1))))
    return np.concatenate(outs, axis=0), res


def kernel(**inputs) -> np.ndarray:
    out, _ = run(inputs, trace=False)
    return out


# revision 13
# speedup vs baseline: 1.2061x; 1.2061x over previous
"""Trainium2 Bass kernel for nn_AutoregressiveRoutingHead.

Model (per batch row b):
    tok_in = [START, tgt[0..6]]                       # teacher forcing, START=5
    x_t    = emb[tok_in[t]]                           # (HID,)
    gi     = x_t @ W_ih.T + b_ih                      # (768,)
    gh     = h @ W_hh.T + b_hh                        # (768,)
    r = sigmoid(gi_r + gh_r); z = sigmoid(gi_z + gh_z)
    n = tanh(gi_n + r * gh_n)
    h' = (1-z)*n + z*h = n - z*(n - h)
    logits_t = h' @ W_out.T + b_out                   # (5,)

Strategy: pure data parallel over batch (65536 -> 8 x 8192). On each core the
hidden state lives TRANSPOSED (latent dim on partitions, batch on the free dim)
so the recurrence matmul needs no per-step transposes. The embedding gather is
a K=8 onehot matmul accumulated into the same PSUM as the recurrence matmul.
Step 0's input is the constant START embedding, folded in as per-partition
activation biases (no matmul at all). Batch is processed in column chunks;
PSUM frames are half-size and double-buffered so one chunk's matmuls overlap
the previous chunk's elementwise work (keeps the PE HAM-warm).
"""

import numpy as np

import concourse.bass as bass
import concourse.mybir as mybir
import concourse.tile as tile
from concourse import bacc, bass_utils

F32 = mybir.dt.float32
AF = mybir.ActivationFunctionType
ALU = mybir.AluOpType

N_CORES = 8
B = 65536
L = 8
LATENT = 256
HID = 128
NTOK = 5
V = NTOK + 1  # vocab incl <start>
START = NTOK
G = 3 * LATENT  # 768 gate rows
KC = LATENT // 128  # 2 contraction chunks
MC = G // 128  # 6 gate-row chunks

B_CORE = B // N_CORES


def build_program(b_core=B_CORE, n_b=512, use_bhh_n=False, use_bout=False, mm="f16"):
    """Build + compile the per-core Bass program (SPMD: same program, 8 cores)."""
    nc = bacc.Bacc("TRN2", target_bir_lowering=False, debug=False)
    if mm == "f32":
        DT = F32
    elif mm == "f32r":
        DT = mybir.dt.float32r
    else:
        DT = mybir.dt.float16  # matmul-input + gate dtype
    n_chunks = b_core // n_b
    n_p = n_b // 128  # 128-row blocks per chunk (for the h0 transpose)

    # ---- DRAM I/O ----------------------------------------------------------
    lat = nc.dram_tensor("lat", [b_core, LATENT], F32, kind="ExternalInput").ap()
    # tokrep[j, t-1, b] = tok_in[b, t] for all j (compare rows 6,7 give 0)
    tokrep = nc.dram_tensor("tokrep", [8, L - 1, b_core], F32, kind="ExternalInput").ap()
    embT = nc.dram_tensor("embT", [HID, V], F32, kind="ExternalInput").ap()
    wihT = nc.dram_tensor("wihT", [HID, G], F32, kind="ExternalInput").ap()
    # row 0: b_ih ; row 1: b_hh with the n-part zeroed (rz part only)
    brows = nc.dram_tensor("brows", [2, G], F32, kind="ExternalInput").ap()
    whhT = nc.dram_tensor("whhT", [LATENT, G], DT, kind="ExternalInput").ap()
    woutT = nc.dram_tensor("woutT", [LATENT, NTOK], DT, kind="ExternalInput").ap()
    ident = nc.dram_tensor("ident", [128, 128], F32, kind="ExternalInput").ap()
    iota = nc.dram_tensor("iota", [8, 1], F32, kind="ExternalInput").ap()
    bhhn = bout = None
    if use_bhh_n:
        bhhn = nc.dram_tensor("bhhn", [1, LATENT], DT, kind="ExternalInput").ap()
    if use_bout:
        bout = nc.dram_tensor("bout", [1, NTOK], DT, kind="ExternalInput").ap()
    outT = nc.dram_tensor("outT", [L, NTOK, b_core], F32, kind="ExternalOutput").ap()

    with tile.TileContext(nc) as tc:
        with tc.tile_pool(name="singles", bufs=1) as singles, \
             tc.tile_pool(name="lat_in", bufs=2) as lat_pool, \
             tc.tile_pool(name="hpool", bufs=3) as h_pool, \
             tc.tile_pool(name="gates", bufs=2) as g_pool, \
             tc.tile_pool(name="ps_rz", bufs=4, space="PSUM") as ps_rz, \
             tc.tile_pool(name="ps_hn", bufs=2, space="PSUM") as ps_hn, \
             tc.tile_pool(name="ps_in", bufs=2, space="PSUM") as ps_in:

            # ---- constants / weights in SBUF -------------------------------
            id_sb = singles.tile([128, 128], F32, tag="ident")
            nc.sync.dma_start(id_sb, ident)
            whh_sb = singles.tile([128, KC, G], DT, tag="whh")
            nc.sync.dma_start(whh_sb, whhT.rearrange("(kc p) n -> p kc n", p=128))
            wout_sb = singles.tile([128, KC, NTOK], DT, tag="wout")
            nc.sync.dma_start(wout_sb, woutT.rearrange("(kc p) n -> p kc n", p=128))
            embT_sb = singles.tile([HID, V], F32, tag="embT")
            nc.sync.dma_start(embT_sb, embT)
            wih_sb = singles.tile([HID, G], F32, tag="wih")
            nc.sync.dma_start(wih_sb, wihT)
            brows_sb = singles.tile([2, G], F32, tag="brows")
            nc.sync.dma_start(brows_sb, brows)
            ones2 = singles.tile([2, V], F32, tag="ones2")
            nc.vector.memset(ones2, 1.0)
            iota_sb = singles.tile([8, 1], F32, tag="iota")
            nc.sync.dma_start(iota_sb, iota)
            bhhn_sb = bout_sb = ones_row = None
            if use_bhh_n or use_bout:
                ones_row = singles.tile([1, n_b], DT, tag="ones_row")
                nc.vector.memset(ones_row, 1.0)
            if use_bhh_n:
                bhhn_sb = singles.tile([1, LATENT], DT, tag="bhhn")
                nc.sync.dma_start(bhhn_sb, bhhn)
            if use_bout:
                bout_sb = singles.tile([1, NTOK], DT, tag="bout")
                nc.sync.dma_start(bout_sb, bout)

            # gi table: gi[v, :] = emb[v] @ W_ih.T + b_ih (+ b_hh on rz part)
            # rows 6,7 zero so the per-step gather matmul can use K=8.
            gi_sb = singles.tile([8, G], DT, tag="gi")
            nc.vector.memset(gi_sb, 0.0)
            gp_a = ps_rz.tile([V, 512], F32, tag="rz")
            nc.tensor.matmul(gp_a, lhsT=embT_sb, rhs=wih_sb[:, 0:512], start=True, stop=False)
            nc.tensor.matmul(gp_a, lhsT=ones2, rhs=brows_sb[:, 0:512], start=False, stop=True)
            gp_b = ps_in.tile([V, G - 512], F32, tag="in")
            nc.tensor.matmul(gp_b, lhsT=embT_sb, rhs=wih_sb[:, 512:G], start=True, stop=False)
            nc.tensor.matmul(gp_b, lhsT=ones2[0:1], rhs=brows_sb[0:1, 512:G], start=False, stop=True)
            nc.any.tensor_copy(gi_sb[0:V, 0:512], gp_a)
            nc.any.tensor_copy(gi_sb[0:V, 512:G], gp_b)
            gi32_sb = gi_sb
            if DT != F32:
                gi32_sb = singles.tile([V, G], F32, tag="gi32")
                nc.any.tensor_copy(gi32_sb[:, 0:512], gp_a)
                nc.any.tensor_copy(gi32_sb[:, 512:G], gp_b)

            # transposed gi chunks (for step-0 constant-START biases)
            giT_sb = singles.tile([128, MC, V], F32, tag="giT")
            for m in range(MC):
                tp = ps_hn.tile([128, V], F32, tag="hn")
                nc.tensor.transpose(tp, gi32_sb[0:V, m * 128:(m + 1) * 128], id_sb[0:V, 0:V])
                nc.any.tensor_copy(giT_sb[:, m, :], tp)

            # ---- main loop: process chunks in PAIRS, steps interleaved,
            # with parity-split PSUM tags (4 banks per parity) so one chunk's
            # matmuls overlap the partner chunk's elementwise chain. ----------
            def chunk_prologue(c, par):
                cs = slice(c * n_b, (c + 1) * n_b)
                lat_sb = lat_pool.tile([128, n_p, LATENT], F32, tag=f"lat{par}",
                                       name="lat_sb")
                nc.sync.dma_start(lat_sb, lat[cs, :].rearrange("(q p) d -> p q d", p=128))
                tokc = lat_pool.tile([8, L - 1, n_b], F32, tag=f"tokc{par}", name="tokc")
                nc.sync.dma_start(tokc, tokrep[:, :, cs])
                oh_c = lat_pool.tile([8, L - 1, n_b], DT, tag=f"ohc{par}", name="oh_c")
                nc.vector.tensor_scalar(oh_c, tokc, iota_sb, None, op0=ALU.is_equal)
                h = h_pool.tile([128, KC, n_b], DT, tag=f"h{par}", name="h0")
                for k in range(KC):
                    for q in range(n_p):
                        tp = ps_hn.tile([128, 128], F32, tag="hn", name="tp")
                        nc.tensor.transpose(tp, lat_sb[:, q, k * 128:(k + 1) * 128], id_sb)
                        nc.vector.tensor_copy(h[:, k, q * 128:(q + 1) * 128], tp)
                return cs, oh_c, h

            def step_body(t, par, cs, oh_c, h):
                # ---- matmuls into PSUM: one rotating bank per gate chunk ----
                rz_ps = [ps_rz.tile([128, n_b], F32, tag="rz", name=f"rz{m}")
                         for m in range(4)]
                hn_ps = [ps_hn.tile([128, n_b], F32, tag="hn", name=f"hn{j}")
                         for j in range(2)]
                in_ps = None
                if t > 0:
                    in_ps = [ps_in.tile([128, n_b], F32, tag="in", name=f"in{j}")
                             for j in range(2)]
                oh = None if t == 0 else oh_c[:, t - 1, :]

                # gi matmuls lead each bank's accumulation group (they only
                # need the onehot, not h'); the partner chunk keeps the PE fed
                # during this chunk's elementwise chain.
                if t > 0:
                    for j in range(2):
                        nc.tensor.matmul(
                            in_ps[j], lhsT=gi_sb[:, (4 + j) * 128:(5 + j) * 128],
                            rhs=oh, start=True, stop=True)
                for m in range(MC):
                    tgt = rz_ps[m] if m < 4 else hn_ps[m - 4]
                    has_gi = (t > 0) and (m < 4)
                    extra_b = (m >= 4) and use_bhh_n
                    if has_gi:
                        nc.tensor.matmul(
                            tgt, lhsT=gi_sb[:, m * 128:(m + 1) * 128],
                            rhs=oh, start=True, stop=False)
                    for k in range(KC):
                        nc.tensor.matmul(
                            tgt,
                            lhsT=whh_sb[:, k, m * 128:(m + 1) * 128],
                            rhs=h[:, k, :],
                            start=(k == 0) and not has_gi,
                            stop=(k == KC - 1) and not extra_b)
                    if extra_b:
                        nc.tensor.matmul(
                            tgt, lhsT=bhhn_sb[:, (m - 4) * 128:(m - 3) * 128],
                            rhs=ones_row, start=False, stop=True)

                # ---- elementwise ----
                rz_sig = g_pool.tile([128, 4, n_b], DT, tag=f"rz_sig{par}", name="rz_sig")
                for m in range(4):
                    nc.scalar.activation(
                        rz_sig[:, m, :], rz_ps[m], AF.Sigmoid,
                        bias=(giT_sb[:, m, START:START + 1] if t == 0 else 0.0))
                r = rz_sig[:, 0:2, :]
                z = rz_sig[:, 2:4, :]
                p = g_pool.tile([128, 2, n_b], DT, tag=f"p{par}", name="p")
                for j in range(2):
                    nc.vector.tensor_mul(p[:, j, :], r[:, j, :], hn_ps[j])
                npre = g_pool.tile([128, 2, n_b], DT, tag=f"npre{par}", name="npre")
                if t == 0:
                    for j in range(2):
                        nc.vector.tensor_scalar_add(
                            npre[:, j, :], p[:, j, :], giT_sb[:, 4 + j, START:START + 1])
                else:
                    for j in range(2):
                        nc.vector.tensor_add(npre[:, j, :], p[:, j, :], in_ps[j])
                nt = g_pool.tile([128, 2, n_b], DT, tag=f"nt{par}", name="nt")
                nc.scalar.activation(nt, npre, AF.Tanh)

                h_new = h_pool.tile([128, KC, n_b], DT, tag=f"h{par}", name="h_new")
                d = g_pool.tile([128, 2, n_b], DT, tag=f"d{par}", name="d")
                e = g_pool.tile([128, 2, n_b], DT, tag=f"e{par}", name="e")
                # h' = n - z*(n - h)
                nc.vector.tensor_tensor(d, nt, h, ALU.subtract)
                nc.vector.tensor_mul(e, z, d)
                nc.vector.tensor_tensor(h_new, nt, e, ALU.subtract)

                # ---- logits ----
                lg = ps_hn.tile([NTOK, n_b], F32, tag="hn", name="lg")
                for k in range(KC):
                    nc.tensor.matmul(
                        lg, lhsT=wout_sb[:, k, :], rhs=h_new[:, k, :],
                        start=(k == 0), stop=(k == KC - 1) and not use_bout)
                if use_bout:
                    nc.tensor.matmul(lg, lhsT=bout_sb, rhs=ones_row, start=False, stop=True)
                lg_sb = g_pool.tile([NTOK, n_b], F32, tag=f"lg{par}", name="lg_sb")
                nc.scalar.copy(lg_sb, lg)
                nc.sync.dma_start(outT[t, :, cs], lg_sb)
                return h_new

            for base in range(0, n_chunks, 2):
                pars = list(range(min(2, n_chunks - base)))
                states = [chunk_prologue(base + par, par) for par in pars]
                for t in range(L):
                    for par in pars:
                        cs, oh_c, h = states[par]
                        h_new = step_body(t, par, cs, oh_c, h)
                        states[par] = (cs, oh_c, h_new)

    nc.compile()
    return nc


def make_in_maps(latent_context, target_sequence, emb_table, W_ih, W_hh,
                 b_ih, b_hh, W_out, b_out, b_core=B_CORE, mm="f16"):
    """Shard + lay out the inputs for each core. Layout-only host transforms."""
    lat = np.ascontiguousarray(np.asarray(latent_context, dtype=np.float32))
    tok = np.asarray(target_sequence).astype(np.float32)
    embT = np.ascontiguousarray(np.asarray(emb_table, dtype=np.float32).T)
    wihT = np.ascontiguousarray(np.asarray(W_ih, dtype=np.float32).T)
    wdt = np.float32 if mm in ("f32", "f32r") else np.float16
    whhT = np.ascontiguousarray(np.asarray(W_hh, dtype=np.float32).T.astype(wdt))
    woutT = np.ascontiguousarray(np.asarray(W_out, dtype=np.float32).T.astype(wdt))
    b_ih = np.asarray(b_ih, dtype=np.float32)
    b_hh = np.asarray(b_hh, dtype=np.float32)
    b_out = np.asarray(b_out, dtype=np.float32)

    brows = np.zeros((2, G), np.float32)
    brows[0] = b_ih
    brows[1, :512] = b_hh[:512]  # n-part of b_hh handled separately
    ident = np.eye(128, dtype=np.float32)
    iota = np.arange(8, dtype=np.float32).reshape(-1, 1)

    n_cores_eff = lat.shape[0] // b_core
    in_maps = []
    for i in range(n_cores_eff):
        sl = slice(i * b_core, (i + 1) * b_core)
        # tokrep[j, t-1, b] = tok_in[b, t] (same for all j)
        tokrep = np.broadcast_to(tok[sl, :L - 1].T[None, :, :], (8, L - 1, b_core))
        m = {
            "lat": lat[sl],
            "tokrep": np.ascontiguousarray(tokrep),
            "embT": embT,
            "wihT": wihT,
            "brows": brows,
            "whhT": whhT,
            "woutT": woutT,
            "ident": ident,
            "iota": iota,
        }
        if np.any(b_hh[512:]):
            m["bhhn"] = np.ascontiguousarray(b_hh[512:].reshape(1, LATENT).astype(wdt))
        if np.any(b_out):
            m["bout"] = np.ascontiguousarray(b_out.reshape(1, NTOK).astype(wdt))
        in_maps.append(m)
    return in_maps


_PROGRAM_CACHE = {}


def _get_program(b_core, use_bhh_n, use_bout, mm):
    key = (b_core, use_bhh_n, use_bout, mm)
    if key not in _PROGRAM_CACHE:
        _PROGRAM_CACHE[key] = build_program(
            b_core=b_core, use_bhh_n=use_bhh_n, use_bout=use_bout, mm=mm)
    return _PROGRAM_CACHE[key]


def run(inputs, trace=False, b_core=B_CORE, mm="f16"):
    in_maps = make_in_maps(b_core=b_core, mm=mm, **inputs)
    use_bhh_n = "bhhn" in in_maps[0]
    use_bout = "bout" in in_maps[0]
    nc = _get_program(b_core, use_bhh_n, use_bout, mm)
    core_ids = list(range(len(in_maps)))
    res = bass_utils.run_bass_kernel_spmd(nc, in_maps, core_ids, trace=trace)
    outs = []
    for i in core_ids:
        o = res.results[i]["outT"]  # (L, NTOK, b_core)
        outs.append(np.ascontiguousarray(np.transpose(o, (2, 0, 1))))
    return np.concatenate(outs, axis=0), res


def kernel(**inputs) -> np.ndarray:
    out, _ = run(inputs, trace=False)
    return out
